# revision 1
# baseline (speedup 1.0000x reference)
"""MultiHeadSelfAttention Trainium2 Bass kernel, 8-core SPMD.

Reference:
  q,k,v = einsum('bnd,hkd->bhnk', x, W_{q,k,v});  s = q k^T / sqrt(dk)
  p = softmax(s); out = (p v).transpose -> [B,N,H*DK]; out @ Wo^T + bo

Sharding: head-pair per core (core c owns heads 2c, 2c+1, all batches).
Each core computes a partial output projection over its 128 d-columns of
Wo; host sums the 8 partials and adds the bias.

Numerics: matmuls run as float32r (fp22 operand reads, fp32 PSUM accum).
Softmax row-max comes from a bf16 scores pass ([q,m] orientation, heads
packed in PE row groups) reduced on DVE via tensor_scalar(op1=min) accum;
the -max is folded into the f32r S^T pass as a 65th contraction row, so
exp needs no per-q bias. Denominators come from a ones column appended to
V. All this was validated numerically on host (rel err ~2.3e-3 vs fp32).
"""
import sys

sys.path.insert(0, "/opt/trn_rl_repo")

import numpy as np

import concourse.bass as bass
import concourse.mybir as mybir
import concourse.tile as tile
from concourse import bacc
from concourse.bass_utils import run_bass_kernel_spmd
from concourse.masks import make_identity

B, N, D = 4, 2048, 1024
H, DK = 16, 64
NCORES = 8
HPC = H // NCORES          # heads per core = 2
DP = HPC * DK              # d-slice per core = 128
SCALE = 1.0 / float(np.sqrt(DK))

F32 = mybir.dt.float32
F32R = mybir.dt.float32r
BF16 = mybir.dt.bfloat16

NQT = N // 128             # 16 q tiles per head
NMC = N // 128             # 16 m chunks per head
NHALF = N // 1024          # 2 halves (1024-wide)
PREP_PRIO_OFFSET = 250     # scheduler hoist distance for next-batch prep


def r(ap):
    return ap.bitcast(F32R)


def build_program():
    nc = bacc.Bacc("TRN2", target_bir_lowering=False, debug=False,
                   enable_asserts=False, num_devices=NCORES)

    xT_d = nc.dram_tensor("xT", [B, D, N], F32, kind="ExternalInput")
    wq_d = nc.dram_tensor("wq", [D, DP], F32, kind="ExternalInput")
    wk_d = nc.dram_tensor("wk", [D, DP], F32, kind="ExternalInput")
    wv_d = nc.dram_tensor("wv", [D, DP], F32, kind="ExternalInput")
    wo_d = nc.dram_tensor("wo", [DP, D], F32, kind="ExternalInput")
    ones_d = nc.dram_tensor("ones", [128, N], F32, kind="ExternalInput")
    part_d = nc.dram_tensor("partial", [B, N, D], F32, kind="ExternalOutput")

    with tile.TileContext(nc) as tc:
        build_tile_kernel(nc, tc, xT_d, wq_d, wk_d, wv_d, wo_d, ones_d, part_d)
    nc.compile()
    return nc


def build_tile_kernel(nc, tc, xT_d, wq_d, wk_d, wv_d, wo_d, ones_d, part_d):
    from contextlib import ExitStack
    ctx = ExitStack()
    with ctx:
        # ---- persistent tiles ----
        wpool = ctx.enter_context(tc.tile_pool(name="w", bufs=1))
        # weights stored chunk-major along free dim: [128, 8*128]
        w_sb = {}
        for name, dram in (("wq", wq_d), ("wk", wk_d), ("wv", wv_d)):
            t = wpool.tile([128, D // 128 * DP], F32R, tag=name)
            nc.sync.dma_start(
                out=t[:].rearrange("p (c m) -> p c m", m=DP),
                in_=dram.ap().rearrange("(c p) m -> p c m", p=128).bitcast(F32R),
            )
            w_sb[name] = t
        wo_sb = wpool.tile([DP, D], F32R, tag="wo")
        nc.sync.dma_start(out=wo_sb[:], in_=wo_d.ap()[:].bitcast(F32R))
        id_sb = wpool.tile([128, 128], F32, tag="ident")
        make_identity(nc, id_sb[:])
        ones_sb = wpool.tile([1, 128], F32R, tag="onesrow")
        nc.sync.dma_start(out=ones_sb[:],
                          in_=ones_d.ap()[0:1, 0:128].bitcast(F32R))

        # ---- pools ----
        # PSUM: ps_main 3x2 banks (S^T: oa+st+st; outproj; normalize) and
        # ps_prep 2x1 banks (projection slices, v-transposes, S~ tiles) so
        # next-batch prep can run concurrently with this batch's S^T phase.
        xt_pool = ctx.enter_context(tc.tile_pool(name="xt", bufs=9))
        ps_main = ctx.enter_context(tc.tile_pool(name="psm", bufs=3, space="PSUM"))
        ps_prep = ctx.enter_context(tc.tile_pool(name="pssq", bufs=2, space="PSUM"))
        augp = ctx.enter_context(tc.tile_pool(name="aug", bufs=6))
        bfp = ctx.enter_context(tc.tile_pool(name="qkbf", bufs=3))
        vsbp = ctx.enter_context(tc.tile_pool(name="vsb", bufs=1))
        vaugp = ctx.enter_context(tc.tile_pool(name="vaug", bufs=2))
        pp = ctx.enter_context(tc.tile_pool(name="psb", bufs=2))
        attp = ctx.enter_context(tc.tile_pool(name="att", bufs=2))
        tmpp = ctx.enter_context(tc.tile_pool(name="tmp", bufs=2))
        scrp = ctx.enter_context(tc.tile_pool(name="scr", bufs=3))
        nmp = ctx.enter_context(tc.tile_pool(name="nm", bufs=4))
        qkfp = ctx.enter_context(tc.tile_pool(name="qkf", bufs=2))
        outp = ctx.enter_context(tc.tile_pool(name="out", bufs=2))

        def proj_all(b, evacs):
            """Project q,k,v via 1-bank psum slices; x half-chunks are
            loaded once and reused by all three tensors."""
            for half in range(NHALF):
                xts = []
                for ch in range(8):
                    xt = xt_pool.tile([128, 1024], F32R, tag="xt")
                    nc.sync.dma_start(
                        out=xt[:],
                        in_=xT_d.ap()[b, ch * 128:(ch + 1) * 128,
                                      half * 1024:(half + 1) * 1024
                                      ].bitcast(F32R),
                    )
                    xts.append(xt)
                for tname, evac in evacs:
                    for ns in range(2):
                        psq = ps_prep.tile([128, 512], F32, tag="sq",
                                           name="prj")
                        for ch in range(8):
                            nc.tensor.matmul(
                                psq[:],
                                r(w_sb[tname][:, ch * DP:(ch + 1) * DP]),
                                r(xts[ch][:, ns * 512:(ns + 1) * 512]),
                                start=(ch == 0), stop=(ch == 7),
                            )
                        evac(psq, half, ns)

        def sweep_head(h, q_bf, k_bf, q_aug):
            """bf16 scores for head h + row-max -> q_aug row 64."""
            hs = slice(h * DK, (h + 1) * DK)
            negmax = nmp.tile([128, 32], F32, tag="nm", name=f"negmax{h}")
            nc.vector.memset(negmax[:, NQT:], 0.0)
            for qt in range(NQT):
                parts = nmp.tile([128, 4], F32, tag="nmparts")
                for ms in range(4):
                    sp = ps_prep.tile([128, 512], F32, tag="sq", name="sq")
                    nc.tensor.matmul(
                        sp[:],
                        q_bf[hs, qt * 128:(qt + 1) * 128],
                        k_bf[hs, ms * 512:(ms + 1) * 512],
                        start=True, stop=True,
                    )
                    scr = scrp.tile([128, 512], BF16, tag="scr")
                    if ms != 3:
                        # DVE: fused negate + min-accum straight from PSUM
                        nc.vector.tensor_scalar(
                            scr[:], sp[:], -1.0, None,
                            mybir.AluOpType.mult, mybir.AluOpType.min,
                            accum_out=parts[:, ms:ms + 1],
                        )
                    else:
                        # ACT (idle in this phase) stages bf16 to SBUF so
                        # the DVE reduce runs in 4x bf16-SBUF mode instead
                        # of the 1x fp32-PSUM path
                        nc.scalar.activation(
                            scr[:], sp[:],
                            mybir.ActivationFunctionType.Copy,
                            bias=0.0, scale=-1.0)
                        scr2 = scrp.tile([128, 512], BF16, tag="scr",
                                         name="scr2")
                        nc.vector.tensor_scalar(
                            scr2[:], scr[:], 0.0, None,
                            mybir.AluOpType.add, mybir.AluOpType.min,
                            accum_out=parts[:, ms:ms + 1],
                        )
                nc.vector.tensor_tensor(parts[:, 0:1], parts[:, 0:1],
                                        parts[:, 1:2], mybir.AluOpType.min)
                nc.vector.tensor_tensor(parts[:, 2:3], parts[:, 2:3],
                                        parts[:, 3:4], mybir.AluOpType.min)
                nc.vector.tensor_tensor(negmax[:, qt:qt + 1], parts[:, 0:1],
                                        parts[:, 2:3], mybir.AluOpType.min)
            # [128,16] -max columns -> row via DVE 32x32 transposes + one DMA
            nm_t = nmp.tile([32, 128], F32, tag="nmt", name=f"nmt{h}")
            for i in range(4):
                nc.vector.transpose(nm_t[0:32, 32 * i:32 * i + 32],
                                    negmax[32 * i:32 * i + 32, 0:32])
            nc.gpsimd.dma_start(out=q_aug[64:65, :],
                                in_=nm_t[0:NQT, :].bitcast(F32R))

        def emit_outproj(b, att):
            for nt in range(N // 128):
                op = ps_main.tile([128, 1024], F32, tag="ps2b", name="op")
                for es in range(2):
                    nc.tensor.matmul(
                        op[:, es * 512:(es + 1) * 512],
                        r(att[:, nt * 128:(nt + 1) * 128]),
                        r(wo_sb[:, es * 512:(es + 1) * 512]),
                        start=True, stop=True,
                    )
                ostg = outp.tile([128, 1024], F32, tag="ostg")
                nc.scalar.copy(ostg[:], op[:])
                nc.sync.dma_start(
                    out=part_d.ap()[b, nt * 128:(nt + 1) * 128, :],
                    in_=ostg[:],
                )

        prev_att = None
        for b in range(B):
            # ======== prep block: pulled into previous batch's S^T ========
            with tc.high_priority(offset=PREP_PRIO_OFFSET if b > 0 else 0):
                q_aug = [augp.tile([65, N], F32R, tag="aug", name=f"qaug{h}")
                         for h in range(HPC)]
                k_aug = [augp.tile([65, N], F32R, tag="aug", name=f"kaug{h}")
                         for h in range(HPC)]
                q_bf = bfp.tile([128, N], BF16, tag="qkbf")
                k_bf = bfp.tile([128, N], BF16, tag="qkbf")
                q_f32 = qkfp.tile([128, N], F32, tag="qkf")
                k_f32 = qkfp.tile([128, N], F32, tag="qkf")
                v_sb = vsbp.tile([128, N], F32, tag="vsb")

                def evac_qk(aug0, f32stage, bf):
                    def evac(psq, half, ns):
                        sl = slice(half * 1024 + ns * 512,
                                   half * 1024 + (ns + 1) * 512)
                        # head 0 straight to its aug tile; head 1 staged then
                        # partition-shift DMA'd
                        nc.scalar.copy(aug0[0:64, sl], psq[0:64, :])
                        nc.scalar.copy(f32stage[64:128, sl], psq[64:128, :])
                        nc.vector.tensor_copy(bf[:, sl], psq[:])
                    return evac

                def evac_v(psq, half, ns):
                    sl = slice(half * 1024 + ns * 512,
                               half * 1024 + (ns + 1) * 512)
                    nc.scalar.copy(v_sb[:, sl], psq[:])

                proj_all(b, [("wq", evac_qk(q_aug[0], q_f32, q_bf)),
                             ("wk", evac_qk(k_aug[0], k_f32, k_bf)),
                             ("wv", evac_v)])
                nc.sync.dma_start(out=q_aug[1][0:64, :],
                                  in_=q_f32[64:128, :].bitcast(F32R))
                nc.sync.dma_start(out=k_aug[1][0:64, :],
                                  in_=k_f32[64:128, :].bitcast(F32R))
                for h in range(HPC):
                    nc.sync.dma_start(out=k_aug[h][64:65, :],
                                      in_=ones_d.ap()[0:1, :].bitcast(F32R))

                # v transpose -> v_aug chunks [v^T(64) | ones(64)]
                v_aug = [vaugp.tile([128, NMC * 128], F32R, tag="vaug",
                                    name=f"vaug{h}") for h in range(HPC)]
                for h in range(HPC):
                    hs = slice(h * DK, (h + 1) * DK)
                    nc.sync.dma_start(
                        out=v_aug[h][:].rearrange(
                            "p (c w) -> p c w", w=128)[:, :, DK:],
                        in_=ones_d.ap()[:, 0:NMC * DK].rearrange(
                            "p (c w) -> p c w", w=DK).bitcast(F32R))
                    for g in range(4):
                        vt_ps = ps_prep.tile([128, 512], F32, tag="sq",
                                             name="vt_ps")
                        for j in range(4):
                            mc = g * 4 + j
                            nc.tensor.transpose(
                                vt_ps[:, j * 128:j * 128 + DK],
                                v_sb[hs, mc * 128:(mc + 1) * 128],
                                id_sb[hs, hs])
                        nc.scalar.copy(
                            v_aug[h][:].rearrange(
                                "p (c w) -> p c w",
                                w=128)[:, g * 4:(g + 1) * 4, 0:DK],
                            vt_ps[:].rearrange("p (c w) -> p c w",
                                               w=128)[:, :, 0:DK])

                # S~ + maxes for head 0 (head 1 overlaps S^T(h0) later)
                sweep_head(0, q_bf, k_bf, q_aug[0])

            # ================== main block ==================
            sweep_head(1, q_bf, k_bf, q_aug[1])

            if prev_att is not None:
                emit_outproj(b - 1, prev_att)
                prev_att = None

            # ======== S^T + exp + pv + normalize per (head, q-half) =======
            att = attp.tile([128, N], F32R, tag="att")
            for h in range(HPC):
                for qh in range(NHALF):
                    qsl = slice(qh * 1024, (qh + 1) * 1024)
                    oa = ps_main.tile([128, 1024], F32, tag="ps2b", name="oa")
                    for mc in range(NMC):
                        st = ps_main.tile([128, 1024], F32, tag="ps2b",
                                          name="st")
                        for qs in range(2):
                            nc.tensor.matmul(
                                st[:, qs * 512:(qs + 1) * 512],
                                r(k_aug[h][:, mc * 128:(mc + 1) * 128]),
                                r(q_aug[h][:, qh * 1024 + qs * 512:
                                           qh * 1024 + (qs + 1) * 512]),
                                start=True, stop=True,
                            )
                        p_sb = pp.tile([128, 1024], F32R, tag="psb")
                        nc.scalar.activation(
                            p_sb[:], st[:],
                            mybir.ActivationFunctionType.Exp,
                            bias=0.0, scale=SCALE)
                        for qs in range(2):
                            nc.tensor.matmul(
                                oa[:, qs * 512:(qs + 1) * 512],
                                r(v_aug[h][:, mc * 128:(mc + 1) * 128]),
                                r(p_sb[:, qs * 512:(qs + 1) * 512]),
                                start=(mc == 0), stop=(mc == NMC - 1),
                            )
                    # normalize: att rows = oa[0:64] * (1/denom); denom
                    # replicated on oa[64:128] via the ones columns of v_aug.
                    # Engine-only: DVE recip -> PE rank-1 -> ACT copy -> DVE.
                    recip = tmpp.tile([1, 1024], F32R, tag="recip")
                    with nc.allow_low_precision(reason="f32r is 4-byte"):
                        nc.vector.reciprocal(recip[:], oa[64:65, :])
                    rb_ps = ps_main.tile([128, 1024], F32, tag="ps2b",
                                         name="rb_ps")
                    for qs in range(2):
                        nc.tensor.matmul(
                            rb_ps[0:64, qs * 512:(qs + 1) * 512],
                            r(ones_sb[0:1, 0:64]),
                            r(recip[0:1, qs * 512:(qs + 1) * 512]),
                            start=True, stop=True,
                        )
                    rbc = tmpp.tile([64, 1024], F32, tag="rbc")
                    nc.vector.tensor_copy(rbc[:], rb_ps[0:64, :])
                    if h == 0:
                        nc.vector.tensor_tensor(
                            att[0:64, qsl], oa[0:64, :], rbc[:],
                            mybir.AluOpType.mult)
                    else:
                        atmp = tmpp.tile([64, 1024], F32R, tag="rbc", name="atmp")
                        nc.vector.tensor_tensor(
                            atmp[:], oa[0:64, :], rbc[:],
                            mybir.AluOpType.mult)
                        # partition shift 0-63 -> 64-127 (DMA; gpsimd queue)
                        nc.gpsimd.dma_start(out=att[64:128, qsl], in_=atmp[:])

            prev_att = att
        # final batch's out-projection
        emit_outproj(B - 1, prev_att)


_PROGRAM = None


def _get_program():
    global _PROGRAM
    if _PROGRAM is None:
        _PROGRAM = build_program()
    return _PROGRAM


_ONES = np.ones((128, N), np.float32)


def make_in_maps(x, W_q, W_k, W_v, Wo_w):
    xT = np.ascontiguousarray(np.transpose(
        np.asarray(x, np.float32), (0, 2, 1)))
    in_maps = []
    for c in range(NCORES):
        hs = slice(HPC * c, HPC * (c + 1))
        wq = np.ascontiguousarray(
            np.asarray(W_q[hs], np.float32).reshape(DP, D).T)
        wk = np.ascontiguousarray(
            np.asarray(W_k[hs], np.float32).reshape(DP, D).T)
        wv = np.ascontiguousarray(
            np.asarray(W_v[hs], np.float32).reshape(DP, D).T)
        wo = np.ascontiguousarray(
            np.asarray(Wo_w, np.float32)[:, DP * c:DP * (c + 1)].T)
        in_maps.append({"xT": xT, "wq": wq, "wk": wk, "wv": wv, "wo": wo,
                        "ones": _ONES})
    return in_maps


def kernel(x, W_q, W_k, W_v, Wo_w, Wo_b):
    nc = _get_program()
    in_maps = make_in_maps(x, W_q, W_k, W_v, Wo_w)
    res = run_bass_kernel_spmd(nc, in_maps, list(range(NCORES)))
    out = res.results[0]["partial"].astype(np.float32)
    for c in range(1, NCORES):
        out += res.results[c]["partial"]
    out += np.asarray(Wo_b, np.float32)
    return out

